# revision 24
# baseline (speedup 1.0000x reference)
"""Trainium2 Bass kernel for nn_EstimationGate: out = history_data * gate(node_emb).

Data-parallel over batch across 8 NeuronCores, with an int8 fixed-point data
path (the 2e-2 rel-err budget admits it: worst-case error is ~1 quantization
step ~ max|h|/127 ~ 1.2% of the output scale).

Host side (per call):
  * quantize history_data to int8 with one global scale 127/max|h|
  * transpose each core's shard to node-major [2048 nodes, 192*32] so that
    every SBUF partition row holds exactly one node's data -> the gate
    multiply becomes a per-partition scalar op
  * stage featw = [concat(emb_u, emb_d).T | W1] as one fp16 block and
    bw = [b1 | concat(W2, b2)] as fp16 so the whole gate MLP needs two DMAs

Device side (per core):
  * the full 12.58MB int8 shard is SBUF-resident: 16 x [128, 6144] tiles all
    loaded up front on the sync HWDGE ring, so the load stream never stalls
    on pool reuse and keeps HBM saturated while the gate MLP runs
  * gate MLP in fp16/f32: 4 PE matmuls -> relu on DVE (tensor_scalar
    add-bias-then-max, keeping ACT free), 16 stationary matmuls into one
    PSUM [128,16], one sigmoid -> gcols[:,t] = gate[t*128 + p]. A dummy
    sigmoid right after setup hoists the ~1.3us ACT table load off the
    critical path.
  * per-partition scaling split by measured engine rates between DVE
    (tensor_scalar_mul, dedicated SBUF port) and ACT (activation Copy with
    scale=gate AP); both engines' f32->int8 conversion rounds to nearest
    (HW-probed). Stores ride the gpsimd SWDGE ring.

Roofline: ~25.7MB of HBM traffic per core at the measured ~415 GB/s => ~62us
plus ~10us fixed NEFF pre/postamble.
"""
import numpy as np

import concourse.bass as bass  # noqa: F401
import concourse.tile as tile
from concourse import bacc, mybir
from concourse.bass_utils import run_bass_kernel_spmd

# Problem shape (hardcoded per spec).
N, E, H = 2048, 64, 64
B, T, C = 32, 48, 32
NCORES = 8
B_SH = B // NCORES           # 4 batches per core
BT = B_SH * T                # 192 (b,t) slabs per core
ROW = BT * C                 # 6144 int8 bytes per node row
NTILE = N // 128             # 16 tiles of [128, ROW] per core
HALF = ROW // 2              # 3072-col compute pieces
QUAR = ROW // 4              # 1536-col pieces for the last tile's tail
# DVE/ACT column split from measured int8 rates (DVE ~0.62 cyc/elt @0.96GHz,
# ACT ~1.11 cyc/elt @1.2GHz). GPSIMD compute is NOT used: its int8
# tensor_scalar ucode runs ~20 cyc/elt and its long ops hold the shared
# DVE/GpSimd SBUF port, serializing the vector engine (measured 6x blowup).
DVE_T = 3872                 # of 6144 full-tile columns
DVE_Q = 1024                 # of 1536 quarter columns

F32 = mybir.dt.float32
F16 = mybir.dt.float16
I8 = mybir.dt.int8

_CACHE = {}


def _build_nc():
    nc = bacc.Bacc("TRN2", target_bir_lowering=False, debug=False)

    hist = nc.declare_dram_parameter("hist", [NTILE, 128, ROW], I8, isOutput=False)
    featw = nc.declare_dram_parameter("featw", [2 * E, N + H], F16, isOutput=False)
    b1 = nc.declare_dram_parameter("b1", [H, 1], F32, isOutput=False)
    w2e = nc.declare_dram_parameter("w2e", [H + 1, 1], F16, isOutput=False)
    out = nc.declare_dram_parameter("out", [NTILE, 128, ROW], I8, isOutput=True)

    with tile.TileContext(nc) as tc:
        with (
            tc.tile_pool(name="setup", bufs=1) as setup,
            tc.tile_pool(name="psum_h", bufs=4, space="PSUM") as psum_h,
            tc.tile_pool(name="psum_g", bufs=1, space="PSUM") as psum_g,
            tc.tile_pool(name="main", bufs=NTILE) as main,
        ):
            # small constants first so the dummy sigmoid can issue early
            scratch = setup.tile([1, 1], F32)
            nc.vector.memset(scratch[:], 0.0)
            hiddenE = setup.tile([H + 1, N], F16)
            nc.gpsimd.memset(hiddenE[H : H + 1, :], 1.0)

            fw = setup.tile([2 * E, N + H], F16)
            nc.scalar.dma_start(fw[:], featw[:])
            b1t = setup.tile([H, 1], F32)
            nc.scalar.dma_start(b1t[:], b1[:])
            w2t = setup.tile([H + 1, 1], F16)
            nc.scalar.dma_start(w2t[:], w2e[:])
            # dummy sigmoid: forces the ACT sigmoid table load to happen now,
            # overlapped with the hist loads, not on the gate critical path
            nc.scalar.activation(
                scratch[:], scratch[:], mybir.ActivationFunctionType.Sigmoid
            )

            # the whole shard, resident: 16 up-front loads on the sync ring
            tiles = []
            for t in range(NTILE):
                ht = main.tile([128, ROW], I8, tag="chunk")
                nc.sync.dma_start(ht[:], hist[t])
                tiles.append(ht)

            # ---- gate MLP ------------------------------------------------
            # hiddenE rows 0:64 = relu(W1.T @ featT + b1) on PE + DVE;
            # row 64 = 1.0 so the w2e matmul adds b2.
            for q in range(4):
                hp = psum_h.tile([H, 512], F32, tag="hp")
                nc.tensor.matmul(
                    hp[:], fw[:, N : N + H], fw[:, q * 512 : (q + 1) * 512],
                    start=True, stop=True,
                )
                nc.vector.tensor_scalar(
                    hiddenE[0:H, q * 512 : (q + 1) * 512], hp[:],
                    b1t[:], 0.0,
                    mybir.AluOpType.add, mybir.AluOpType.max,
                )

            # gcols[p, t] = sigmoid(hidden[t*128+p] . W2 + b2)
            gp = psum_g.tile([128, NTILE], F32, tag="gp")
            for t in range(NTILE):
                nc.tensor.matmul(
                    gp[:, t : t + 1],
                    hiddenE[:, t * 128 : (t + 1) * 128],
                    w2t[:],
                    start=True, stop=True,
                )
            gcols = setup.tile([128, NTILE], F32)
            nc.scalar.activation(
                gcols[:], gp[:], mybir.ActivationFunctionType.Sigmoid
            )

            # ---- streaming int8 scale ------------------------------------
            # DVE and ACT split each piece's columns; 768KB stores ride the
            # gpsimd SWDGE ring (own descriptor path, no HWDGE-lane contention
            # with the loads).
            def piece(t, c0, c1, dve_cols):
                ht = tiles[t]
                gk = gcols[:, t : t + 1]
                d1 = c0 + dve_cols
                nc.vector.tensor_scalar_mul(ht[:, c0:d1], ht[:, c0:d1], gk)
                nc.scalar.mul(ht[:, d1:c1], ht[:, d1:c1], gk)
                nc.gpsimd.dma_start(out[t][:, c0:c1], ht[:, c0:c1])

            # full-tile pieces (768KB stores amortize the ~0.65us per-store
            # SWDGE queue cost; smaller stores drain at ~270GB/s vs ~425);
            # the last tile runs in quarters to shorten the tail chain
            for t in range(NTILE - 1):
                piece(t, 0, ROW, DVE_T)
            for s in range(4):
                piece(NTILE - 1, s * QUAR, (s + 1) * QUAR, DVE_Q)

    nc.compile()
    return nc


def _run(inputs, trace=False, trace_kwargs=None):
    if "nc" not in _CACHE:
        _CACHE["nc"] = _build_nc()
    nc = _CACHE["nc"]

    hist = np.asarray(inputs["history_data"], dtype=np.float32)
    s_max = float(np.abs(hist).max())
    if s_max == 0.0:
        s_max = 1.0
    q = np.rint(hist * np.float32(127.0 / s_max)).astype(np.int8)
    q = q.reshape(NCORES, B_SH, T, N, C)

    emb_u = np.asarray(inputs["node_embedding_u"], np.float32)
    emb_d = np.asarray(inputs["node_embedding_d"], np.float32)
    featT = np.concatenate([emb_u, emb_d], axis=1).T          # [128, 2048]
    w1 = np.asarray(inputs["W1"], np.float32)                 # [128, 64]
    featw = np.ascontiguousarray(
        np.concatenate([featT, w1], axis=1).astype(np.float16)
    )
    w2e = np.concatenate(
        [np.asarray(inputs["W2"], np.float32).reshape(H, 1),
         np.asarray(inputs["b2"], np.float32).reshape(1, 1)], axis=0
    ).astype(np.float16)
    common = {
        "featw": featw,
        "b1": np.ascontiguousarray(np.asarray(inputs["b1"], np.float32).reshape(H, 1)),
        "w2e": np.ascontiguousarray(w2e),
    }

    in_maps = []
    for c in range(NCORES):
        hq = np.ascontiguousarray(q[c].transpose(2, 0, 1, 3).reshape(N, ROW))
        in_maps.append({"hist": hq.reshape(NTILE, 128, ROW), **common})

    kw = {}
    if trace:
        kw["trace"] = True
        if trace_kwargs:
            kw["trace_kwargs"] = trace_kwargs
    res = run_bass_kernel_spmd(nc, in_maps, list(range(NCORES)), **kw)

    inv = np.float32(s_max / 127.0)
    out = np.empty((B, T, N, C), np.float32)
    for c in range(NCORES):
        o = res.results[c]["out"].reshape(N, B_SH, T, C).transpose(1, 2, 0, 3)
        np.multiply(o, inv, out=out[c * B_SH : (c + 1) * B_SH])
    return out, res


def kernel(**inputs):
    out, _ = _run(inputs)
    return out


# revision 33
# speedup vs baseline: 1.0116x; 1.0116x over previous
"""Trainium2 Bass kernel for nn_EstimationGate: out = history_data * gate(node_emb).

Data-parallel over batch across 8 NeuronCores, with an int8 fixed-point data
path (the 2e-2 rel-err budget admits it: worst-case error is ~1 quantization
step ~ max|h|/127 ~ 1.2% of the output scale).

Host side (per call):
  * quantize history_data to int8 with one global scale 127/max|h|
  * transpose each core's shard to node-major [2048 nodes, 192*32] so that
    every SBUF partition row holds exactly one node's data -> the gate
    multiply becomes a per-partition scalar op
  * stage featw = [concat(emb_u, emb_d).T | W1] as one fp16 block and
    bw = [b1 | concat(W2, b2)] as fp16 so the whole gate MLP needs two DMAs

Device side (per core):
  * the full 12.58MB int8 shard is SBUF-resident: 16 x [128, 6144] tiles all
    loaded up front on the sync HWDGE ring, so the load stream never stalls
    on pool reuse and keeps HBM saturated while the gate MLP runs
  * gate MLP in fp16/f32: 4 PE matmuls -> relu on DVE (tensor_scalar
    add-bias-then-max, keeping ACT free), 16 stationary matmuls into one
    PSUM [128,16], one sigmoid -> gcols[:,t] = gate[t*128 + p]. A dummy
    sigmoid right after setup hoists the ~1.3us ACT table load off the
    critical path.
  * per-partition scaling split by measured engine rates between DVE
    (tensor_scalar_mul, dedicated SBUF port) and ACT (activation Copy with
    scale=gate AP); both engines' f32->int8 conversion rounds to nearest
    (HW-probed). Stores ride the gpsimd SWDGE ring.

Roofline: ~25.7MB of HBM traffic per core at the measured ~415 GB/s => ~62us
plus ~10us fixed NEFF pre/postamble.
"""
import numpy as np

import concourse.bass as bass  # noqa: F401
import concourse.tile as tile
from concourse import bacc, mybir
from concourse.bass_utils import run_bass_kernel_spmd

# Problem shape (hardcoded per spec).
N, E, H = 2048, 64, 64
B, T, C = 32, 48, 32
NCORES = 8
B_SH = B // NCORES           # 4 batches per core
BT = B_SH * T                # 192 (b,t) slabs per core
ROW = BT * C                 # 6144 int8 bytes per node row
NTILE = N // 128             # 16 tiles of [128, ROW] per core
HALF = ROW // 2              # 3072-col compute pieces
QUAR = ROW // 4              # 1536-col pieces for the last tile's tail
# DVE/ACT column split from measured int8 rates (DVE ~0.62 cyc/elt @0.96GHz,
# ACT ~1.11 cyc/elt @1.2GHz). GPSIMD compute is NOT used: its int8
# tensor_scalar ucode runs ~20 cyc/elt and its long ops hold the shared
# DVE/GpSimd SBUF port, serializing the vector engine (measured 6x blowup).
DVE_T = 3872                 # of 6144 full-tile columns
DVE_Q = 1024                 # of 1536 quarter columns

F32 = mybir.dt.float32
F16 = mybir.dt.float16
I8 = mybir.dt.int8

_CACHE = {}


def _build_nc():
    nc = bacc.Bacc("TRN2", target_bir_lowering=False, debug=False)

    hist = nc.declare_dram_parameter("hist", [NTILE // 2, 128, 2 * ROW], I8, isOutput=False)
    featw = nc.declare_dram_parameter("featw", [2 * E, N + H], F16, isOutput=False)
    b1 = nc.declare_dram_parameter("b1", [H, 1], F32, isOutput=False)
    w2e = nc.declare_dram_parameter("w2e", [H + 1, 1], F16, isOutput=False)
    out = nc.declare_dram_parameter("out", [NTILE, 128, ROW], I8, isOutput=True)

    with tile.TileContext(nc) as tc:
        with (
            tc.tile_pool(name="setup", bufs=1) as setup,
            tc.tile_pool(name="psum_h", bufs=4, space="PSUM") as psum_h,
            tc.tile_pool(name="psum_g", bufs=1, space="PSUM") as psum_g,
            tc.tile_pool(name="main", bufs=NTILE) as main,
        ):
            # small constants first so the dummy sigmoid can issue early
            scratch = setup.tile([1, 1], F32)
            nc.vector.memset(scratch[:], 0.0)
            hiddenE = setup.tile([H + 1, N], F16)
            nc.gpsimd.memset(hiddenE[H : H + 1, :], 1.0)

            fw = setup.tile([2 * E, N + H], F16)
            nc.scalar.dma_start(fw[:], featw[:])
            b1t = setup.tile([H, 1], F32)
            nc.scalar.dma_start(b1t[:], b1[:])
            w2t = setup.tile([H + 1, 1], F16)
            nc.scalar.dma_start(w2t[:], w2e[:])
            # dummy sigmoid: forces the ACT sigmoid table load to happen now,
            # overlapped with the hist loads, not on the gate critical path
            nc.scalar.activation(
                scratch[:], scratch[:], mybir.ActivationFunctionType.Sigmoid
            )

            # the whole shard, resident: 8 double-tile loads (1.57MB each) so
            # all of them fit the 8 HWDGE in-flight lanes at once -- the full
            # load stream queues up front with no trigger stalls
            tiles = []
            for k in range(NTILE // 2):
                dt = main.tile([128, 2 * ROW], I8, tag="chunk")
                nc.sync.dma_start(dt[:], hist[k])
                tiles.append((dt, 0))
                tiles.append((dt, ROW))

            # ---- gate MLP ------------------------------------------------
            # hiddenE rows 0:64 = relu(W1.T @ featT + b1) on PE + DVE;
            # row 64 = 1.0 so the w2e matmul adds b2.
            for q in range(4):
                hp = psum_h.tile([H, 512], F32, tag="hp")
                nc.tensor.matmul(
                    hp[:], fw[:, N : N + H], fw[:, q * 512 : (q + 1) * 512],
                    start=True, stop=True,
                )
                nc.vector.tensor_scalar(
                    hiddenE[0:H, q * 512 : (q + 1) * 512], hp[:],
                    b1t[:], 0.0,
                    mybir.AluOpType.add, mybir.AluOpType.max,
                )

            # gcols[p, t] = sigmoid(hidden[t*128+p] . W2 + b2)
            gp = psum_g.tile([128, NTILE], F32, tag="gp")
            for t in range(NTILE):
                nc.tensor.matmul(
                    gp[:, t : t + 1],
                    hiddenE[:, t * 128 : (t + 1) * 128],
                    w2t[:],
                    start=True, stop=True,
                )
            gcols = setup.tile([128, NTILE], F32)
            nc.scalar.activation(
                gcols[:], gp[:], mybir.ActivationFunctionType.Sigmoid
            )

            # ---- streaming int8 scale ------------------------------------
            # DVE and ACT split each piece's columns; 768KB stores ride the
            # gpsimd SWDGE ring (own descriptor path, no HWDGE-lane contention
            # with the loads).
            def piece(t, c0, c1, dve_cols):
                dt, off = tiles[t]
                gk = gcols[:, t : t + 1]
                b0, b1_, d1 = off + c0, off + c1, off + c0 + dve_cols
                nc.vector.tensor_scalar_mul(dt[:, b0:d1], dt[:, b0:d1], gk)
                nc.scalar.mul(dt[:, d1:b1_], dt[:, d1:b1_], gk)
                nc.gpsimd.dma_start(out[t][:, c0:c1], dt[:, b0:b1_])

            # full-tile pieces (768KB stores amortize the ~0.65us per-store
            # SWDGE queue cost; smaller stores drain at ~270GB/s vs ~425);
            # the last tile runs in quarters to shorten the tail chain
            for t in range(NTILE - 1):
                piece(t, 0, ROW, DVE_T)
            for s in range(4):
                piece(NTILE - 1, s * QUAR, (s + 1) * QUAR, DVE_Q)

    nc.compile()
    return nc


def _run(inputs, trace=False, trace_kwargs=None):
    if "nc" not in _CACHE:
        _CACHE["nc"] = _build_nc()
    nc = _CACHE["nc"]

    hist = np.asarray(inputs["history_data"], dtype=np.float32)
    s_max = float(np.abs(hist).max())
    if s_max == 0.0:
        s_max = 1.0
    q = np.rint(hist * np.float32(127.0 / s_max)).astype(np.int8)
    q = q.reshape(NCORES, B_SH, T, N, C)

    emb_u = np.asarray(inputs["node_embedding_u"], np.float32)
    emb_d = np.asarray(inputs["node_embedding_d"], np.float32)
    featT = np.concatenate([emb_u, emb_d], axis=1).T          # [128, 2048]
    w1 = np.asarray(inputs["W1"], np.float32)                 # [128, 64]
    featw = np.ascontiguousarray(
        np.concatenate([featT, w1], axis=1).astype(np.float16)
    )
    w2e = np.concatenate(
        [np.asarray(inputs["W2"], np.float32).reshape(H, 1),
         np.asarray(inputs["b2"], np.float32).reshape(1, 1)], axis=0
    ).astype(np.float16)
    common = {
        "featw": featw,
        "b1": np.ascontiguousarray(np.asarray(inputs["b1"], np.float32).reshape(H, 1)),
        "w2e": np.ascontiguousarray(w2e),
    }

    in_maps = []
    for c in range(NCORES):
        hq = np.ascontiguousarray(q[c].transpose(2, 0, 1, 3).reshape(N, ROW))
        # interleave tile pairs so each double-tile load is one contiguous
        # [128, 2*ROW] block: line p = [tile 2k row p | tile 2k+1 row p]
        hq2 = np.ascontiguousarray(
            hq.reshape(NTILE // 2, 2, 128, ROW).transpose(0, 2, 1, 3)
        )
        in_maps.append({"hist": hq2.reshape(NTILE // 2, 128, 2 * ROW), **common})

    kw = {}
    if trace:
        kw["trace"] = True
        if trace_kwargs:
            kw["trace_kwargs"] = trace_kwargs
    res = run_bass_kernel_spmd(nc, in_maps, list(range(NCORES)), **kw)

    inv = np.float32(s_max / 127.0)
    out = np.empty((B, T, N, C), np.float32)
    for c in range(NCORES):
        o = res.results[c]["out"].reshape(N, B_SH, T, C).transpose(1, 2, 0, 3)
        np.multiply(o, inv, out=out[c * B_SH : (c + 1) * B_SH])
    return out, res


def kernel(**inputs):
    out, _ = _run(inputs)
    return out
